# revision 25
# baseline (speedup 1.0000x reference)
"""HODLR matvec kernel for 8 TRN2 NeuronCores (Bass/Tile).

Sharding: node axis split into 8 contiguous slices of 32768 nodes.
Per core:
  projection  t[l,r,b] = sum_c u[l,c,r] * x[b,c]   (per block, all 8 levels)
              done in two passes: level-pairs (0,1) first, then (2,3),
              so the cross-core collective overlaps the second pass
  tree        combine L7-block partials up to coarser blocks
  A2A         exchange levels 0-2 sibling coefficients across cores
              (sender-side 0/1 masks make the combination core-invariant)
  expansion   corr[b,n] = sum_{l,r} u[l,n,r] * t_sib[l,r,b]
u/x are fed as fp8e4m3 (u scaled by USCALE; host divides the returned
correction by USCALE^2). The expansion runs fp8 DoubleRow matmuls that
contract two level-pairs (K=256) per instruction. Host computes diag*x
in fp32 and adds the device-computed correction.
"""

import os
import sys

sys.path.insert(0, "/opt/trn_rl_repo")

import numpy as np
import ml_dtypes

BF16 = ml_dtypes.bfloat16
FP8 = ml_dtypes.float8_e4m3

B = 64
N = 262144
NCORES = 8
M = N // NCORES          # 32768 nodes per core
R = 64
DEPTH = 8
CH = M // 128            # 256 chunks of 128 nodes
NB7 = M // 1024          # 32 L7 blocks (1024 nodes each)
USCALE = 64.0            # u is fed as u*USCALE in fp8 (e4m3 max finite 240)

_cached = {}


def _build_bass():
    import concourse.bacc as bacc
    import concourse.tile as tile
    import concourse.mybir as mybir
    from contextlib import ExitStack

    BF = mybir.dt.bfloat16
    F8 = mybir.dt.float8e4
    F32 = mybir.dt.float32
    ADD = mybir.AluOpType.add
    MULT = mybir.AluOpType.mult

    nc = bacc.Bacc(
        "TRN2",
        target_bir_lowering=False,
        debug=False,
        enable_asserts=False,
        num_devices=NCORES,
    )

    xt_d = nc.dram_tensor("xt", [128, CH, B], F8, kind="ExternalInput").ap()
    # u packed phase-major: [:, ph, k, :] holds level-pairs (2ph, 2ph+1)
    u_d = nc.dram_tensor("u", [128, 2, CH, 256], F8, kind="ExternalInput").ap()
    ut_d = nc.dram_tensor("ut", [4, 128, M], F8, kind="ExternalInput").ap()
    mA_d = nc.dram_tensor("maskA", [128, 8, B], BF, kind="ExternalInput").ap()
    mB_d = nc.dram_tensor("maskB", [64, 8, B], BF, kind="ExternalInput").ap()
    corr23_d = nc.dram_tensor("corr23", [B, M], F8, kind="ExternalOutput").ap()
    corr01_d = nc.dram_tensor("corr01", [B, M], F8, kind="ExternalOutput").ap()

    with tile.TileContext(nc) as tc, ExitStack() as ctx:
        const = ctx.enter_context(tc.tile_pool(name="const", bufs=1))
        upool = ctx.enter_context(tc.tile_pool(name="upool", bufs=4))
        pp = ctx.enter_context(tc.tile_pool(name="pp", bufs=3, space="PSUM"))
        treep = ctx.enter_context(tc.tile_pool(name="treep", bufs=1))
        statp = ctx.enter_context(tc.tile_pool(name="statp", bufs=1))
        ut23p = ctx.enter_context(tc.tile_pool(name="ut23p", bufs=4))
        ut01p = ctx.enter_context(tc.tile_pool(name="ut01p", bufs=8))
        ep23 = ctx.enter_context(tc.tile_pool(name="ep23", bufs=3, space="PSUM"))
        ep01 = ctx.enter_context(tc.tile_pool(name="ep01", bufs=2, space="PSUM"))
        y23p = ctx.enter_context(tc.tile_pool(name="y23p", bufs=3))
        y01p = ctx.enter_context(tc.tile_pool(name="y01p", bufs=2))
        dram = ctx.enter_context(tc.tile_pool(name="dram", bufs=1, space="DRAM"))

        # All big input streams go on the SYNC queue in priority order:
        # xt -> ph0 u -> ph1 u -> ut.  A single queue spreads across all
        # 16 SDMA engines (full BW) and guarantees the stream order, so
        # the expansion's ut stream starts the moment u finishes instead
        # of after the collective (the baseline's ~45us DMA dead zone).
        xt = const.tile([128, CH, B], F8, tag="xt")
        for xq in range(4):
            nc.sync.dma_start(
                xt[:, 64 * xq : 64 * (xq + 1), :],
                xt_d[:, 64 * xq : 64 * (xq + 1), :],
            )
        mA = const.tile([128, 8, B], BF, tag="mA")
        nc.scalar.dma_start(mA[:], mA_d[:])
        mB = const.tile([64, 8, B], BF, tag="mB")
        nc.scalar.dma_start(mB[:], mB_d[:])
        b_in = dram.tile([192, B], BF, tag="b_in")
        b_out = dram.tile([8, 192, B], BF, tag="b_out", addr_space="Shared")

        # ---------------- projection + per-pair trees ----------------
        # G[(q, sz)][j]: [128, 64] bf16; rows 0:64 -> level 2q, rows
        # 64:128 -> level 2q+1 of the j-th block of `sz` nodes (t^T
        # layout [(l,r), b], scaled by USCALE).
        need_top = {3: 2048, 2: 8192, 1: 32768, 0: 32768}
        G = {}

        # Round-robin PSUM drains / casts across Vector, Scalar(ACT) and
        # scheduler-assigned to keep any single engine off the critical
        # path (the baseline had Vector at ~50% busy).
        _cc = [0]

        def tcopy(i, out, in_, engines="vs"):
            # strictly explicit engines: the `any` scheduler can park ops
            # on gpsimd AHEAD of the collective trigger (strict FIFO),
            # delaying the whole collective by tens of us.
            e = engines[_cc[0] % len(engines)]
            _cc[0] += 1
            if e == "v":
                nc.vector.tensor_copy(out, in_)
            else:
                nc.scalar.copy(out, in_)

        _ca = [0]

        def tadd(i, out, a, b):
            nc.vector.tensor_tensor(out, a, b, op=ADD)

        for ph, qs in ((0, (0, 1)), (1, (2, 3))):
            psums = {}  # (q, j) -> psum tile kept for direct tree combine
            for j in range(NB7):
                if j % 4 == 0:
                    u_t2 = upool.tile(
                        [128, 32, 256], F8, tag="u_in", name=f"u_t{ph}_{j}"
                    )
                    if ph == 0 and j == 0:
                        # split the first tile so MM #1 only waits on a
                        # quarter-tile DMA (earlier PE start)
                        for sq in range(4):
                            nc.sync.dma_start(
                                u_t2[:, 8 * sq : 8 * sq + 8, :],
                                u_d[:, ph, 8 * sq : 8 * sq + 8, :],
                            )
                    else:
                        nc.sync.dma_start(
                            u_t2[:], u_d[:, ph, 8 * j : 8 * j + 32, :]
                        )
                u_t = u_t2[:, 8 * (j % 4) : 8 * (j % 4) + 8, :]
                engset = "vs"
                for qi, q in enumerate(qs):
                    ps = pp.tile([128, B], F32, tag="proj", name=f"ps{q}_{j}")
                    for ki in range(8):
                        k = 8 * j + ki
                        nc.tensor.matmul(
                            ps[:],
                            u_t[:, ki, 128 * qi : 128 * qi + 128],
                            xt[:, k, :],
                            start=(ki == 0),
                            stop=(ki == 7),
                        )
                    if q == 3:
                        # pair 3 needs the 1024-blocks themselves
                        g = treep.tile([128, B], BF, tag=f"G3_1024_{j}")
                        tcopy(j, g[:], ps[:], engset)
                        G.setdefault((3, 1024), []).append(g)
                    else:
                        if j % 2 == 0:
                            # drain even block to SBUF (an op may read at
                            # most one PSUM input)
                            tmp = treep.tile(
                                [128, B], BF, tag=f"T{q}_{j // 2}",
                                name=f"T{q}_{j // 2}",
                            )
                            tcopy(j // 2 + q, tmp[:], ps[:], engset)
                            psums[(q, j)] = tmp
                        else:
                            g2 = treep.tile(
                                [128, B], BF, tag=f"G{q}_2048_{j // 2}"
                            )
                            tadd(
                                j // 2,
                                g2[:],
                                psums.pop((q, j - 1))[:],
                                ps[:],
                            )
                            G.setdefault((q, 2048), []).append(g2)
            # tree up
            for q in qs:
                if q == 3:
                    lst = G[(3, 1024)]
                    nxt = []
                    for j in range(len(lst) // 2):
                        g2 = treep.tile([128, B], BF, tag=f"G3_2048_{j}")
                        tadd(j, g2[:], lst[2 * j][:], lst[2 * j + 1][:])
                        nxt.append(g2)
                    G[(3, 2048)] = nxt
                sz = 2048
                while sz < need_top[q]:
                    lst = G[(q, sz)]
                    nxt = []
                    for j in range(len(lst) // 2):
                        g2 = treep.tile([128, B], BF, tag=f"G{q}_{2 * sz}_{j}")
                        tadd(j + q, g2[:], lst[2 * j][:], lst[2 * j + 1][:])
                        nxt.append(g2)
                    G[(q, 2 * sz)] = nxt
                    sz *= 2

            if ph == 0:
                # ------------- collective (overlaps phase 1) -------------
                # AllGather the raw level-0..2 partials; the receive-side
                # masked combine happens later (on gpsimd), once the CC
                # lands.  The b_in copies are plain contiguous SBUF->DRAM
                # writes (fast); the trigger sits on the gpsimd queue.
                A = G[(0, 32768)][0]
                Bt = G[(1, 32768)][0]
                nc.scalar.dma_start(b_in[0:128, :], A[:])
                nc.scalar.dma_start(b_in[128:192, :], Bt[0:64, :])
                nc.gpsimd.collective_compute(
                    "AllGather",
                    mybir.AluOpType.bypass,
                    replica_groups=[list(range(NCORES))],
                    ins=[b_in.opt()],
                    outs=[b_out.opt()],
                )

        # ---- ut streams on the sync queue: ut23 (levels 4-7) first,
        # then the collective recv DMAs (their HWDGE completion-lane
        # slots land between the two streams, completing right when the
        # statf01 pass needs them), then ut01 (levels 0-3).
        ut23_tiles = []
        for gb in range(8):
            t_ = ut23p.tile([128, 2, 4096], F8, tag="ut23", name=f"ut23_{gb}")
            for ff in range(2):
                nc.sync.dma_start(
                    t_[:, ff, :],
                    ut_d[2 + ff, :, 4096 * gb : 4096 * (gb + 1)],
                )
            ut23_tiles.append(t_)

        # collective receive (16 small fast DMAs; they wait on the CC)
        recvA = statp.tile([128, 8, B], BF, tag="recvA")
        recvB = statp.tile([64, 8, B], BF, tag="recvB")
        for k in range(8):
            nc.scalar.dma_start(recvA[:, k, :], b_out[k, 0:128, :])
            nc.scalar.dma_start(recvB[:, k, :], b_out[k, 128:192, :])

        ut01_tiles = []
        for gb in range(8):
            t_ = ut01p.tile([128, 2, 4096], F8, tag="ut01", name=f"ut01_{gb}")
            for ff in range(2):
                nc.sync.dma_start(
                    t_[:, ff, :], ut_d[ff, :, 4096 * gb : 4096 * (gb + 1)]
                )
            ut01_tiles.append(t_)

        # ---------------- expansion stationaries (fp8, DoubleRow) --------
        # statf[q-pair-fuse] tiles [128, 2, B]: [:, j, :] holds the
        # stationary of level-pair (2*fuse + j); rows 0:64 = t_sib at the
        # even level of that pair, rows 64:128 = at the odd level.
        # statf23 first: it has only local dependencies, so the
        # 23-pass matmuls can fire as soon as ut23 arrives.
        statf23 = []
        for m7 in range(NB7):
            s = statp.tile([128, 2, B], F8, tag=f"sf23_{m7}", name=f"sf23_{m7}")
            m5 = m7 // 4
            tcopy(0, s[0:64, 0, :], G[(2, 8192)][(m5 // 2) ^ 1][0:64, :], "vs")
            tcopy(0, s[64:128, 0, :], G[(2, 4096)][m5 ^ 1][64:128, :], "vs")
            tcopy(0, s[0:64, 1, :], G[(3, 2048)][(m7 // 2) ^ 1][0:64, :], "vs")
            tcopy(0, s[64:128, 1, :], G[(3, 1024)][m7 ^ 1][64:128, :], "vs")
            statf23.append(s)

        # ---- masked combine + statf01 (all on gpsimd: it is idle and
        # everything here waits on the collective anyway) ----
        mskA = statp.tile([128, 8, B], BF, tag="mskA")
        mskB = statp.tile([64, 8, B], BF, tag="mskB")
        nc.gpsimd.tensor_tensor(mskA[:], recvA[:], mA[:], op=MULT)
        nc.gpsimd.tensor_tensor(mskB[:], recvB[:], mB[:], op=MULT)
        nc.gpsimd.tensor_tensor(
            mskA[:, 0:4, :], mskA[:, 0:4, :], mskA[:, 4:8, :], op=ADD
        )
        nc.gpsimd.tensor_tensor(
            mskB[:, 0:4, :], mskB[:, 0:4, :], mskB[:, 4:8, :], op=ADD
        )
        nc.gpsimd.tensor_tensor(
            mskA[:, 0:2, :], mskA[:, 0:2, :], mskA[:, 2:4, :], op=ADD
        )
        nc.gpsimd.tensor_tensor(
            mskB[:, 0:2, :], mskB[:, 0:2, :], mskB[:, 2:4, :], op=ADD
        )
        tallA = statp.tile([128, B], BF, tag="tallA")
        tallB = statp.tile([64, B], BF, tag="tallB")
        nc.gpsimd.tensor_tensor(tallA[:], mskA[:, 0, :], mskA[:, 1, :], op=ADD)
        nc.gpsimd.tensor_tensor(tallB[:], mskB[:, 0, :], mskB[:, 1, :], op=ADD)
        statf01 = []
        for m3 in range(2):
            s = statp.tile([128, 2, B], F8, tag=f"sf01_{m3}", name=f"sf01_{m3}")
            nc.gpsimd.tensor_copy(s[:, 0, :], tallA[:])
            nc.gpsimd.tensor_copy(s[0:64, 1, :], tallB[:])
            nc.gpsimd.tensor_copy(
                s[64:128, 1, :], G[(1, 16384)][m3 ^ 1][64:128, :]
            )
            statf01.append(s)

        # ---------------- expansion pass 1: levels 4-7 ----------------
        # Self-contained accumulation groups (start+stop in one MM), so
        # nothing here ever waits on the collective.
        DR = mybir.MatmulPerfMode.DoubleRow
        for gb in range(8):
            ut_t = ut23_tiles[gb]
            y_sb = y23p.tile([B, 4096], F8, tag="y23")
            for gg in range(8):
                g = 8 * gb + gg
                eps = ep23.tile([B, 512], F32, tag="e23", name=f"e23_{g}")
                sl = slice(512 * gg, 512 * (gg + 1))
                nc.tensor.matmul(
                    eps[:], statf23[g // 2][:], ut_t[:, :, sl],
                    start=True, stop=True, perf_mode=DR,
                )
                tcopy(g, y_sb[:, sl], eps[:])
            nc.scalar.dma_start(
                corr23_d[:, 4096 * gb : 4096 * (gb + 1)], y_sb[:]
            )

        # ---------------- expansion pass 2: levels 0-3 ----------------
        for gb in range(8):
            ut_t = ut01_tiles[gb]
            y_sb = y01p.tile([B, 4096], F8, tag="y01")
            for gg in range(8):
                g = 8 * gb + gg
                eps = ep01.tile([B, 512], F32, tag="e01", name=f"e01_{g}")
                sl = slice(512 * gg, 512 * (gg + 1))
                nc.tensor.matmul(
                    eps[:], statf01[g // 32][:], ut_t[:, :, sl],
                    start=True, stop=True, perf_mode=DR,
                )
                tcopy(g, y_sb[:, sl], eps[:])
            nc.scalar.dma_start(
                corr01_d[:, 4096 * gb : 4096 * (gb + 1)], y_sb[:]
            )

    nc.compile()
    return nc


def _pack_inputs(x, diag, u):
    """Build per-core input maps. x (B,N,1) f32, u (DEPTH,N,R) f32."""
    in_maps = []
    x2 = np.asarray(x).reshape(B, N)
    u3 = np.asarray(u)
    for c in range(NCORES):
        base = c * M
        xsl = x2[:, base : base + M]                      # (B, M)
        usl = u3[:, base : base + M, :] * USCALE          # (8, M, 64)
        xt = np.ascontiguousarray(
            xsl.T.reshape(CH, 128, B).transpose(1, 0, 2)
        ).astype(FP8)                                     # [128, CH, B]
        up = np.ascontiguousarray(
            usl.transpose(1, 0, 2).reshape(M, 512)        # [n, l*64+r]
            .reshape(CH, 128, 2, 256)
            .transpose(1, 2, 0, 3)
        ).astype(FP8)                                     # [128, 2, CH, 256]
        utp = np.ascontiguousarray(
            usl.transpose(0, 2, 1).reshape(4, 128, M)
        ).astype(FP8)                                     # [4, 128, M]
        # masks: mask[d, l] = 1 iff this core c is in the level-l sibling
        # block of destination core d.
        mA = np.zeros((128, 8, B), dtype=BF16)
        mB = np.zeros((64, 8, B), dtype=BF16)
        for d in range(8):
            if (c // 4) == ((d // 4) ^ 1):
                mA[0:64, d, :] = 1.0   # level 0
            if (c // 2) == ((d // 2) ^ 1):
                mA[64:128, d, :] = 1.0  # level 1
            if c == d ^ 1:
                mB[:, d, :] = 1.0       # level 2
        in_maps.append({"xt": xt, "u": up, "ut": utp, "maskA": mA, "maskB": mB})
    return in_maps


last_results = None


def kernel(x, diag, u):
    global last_results
    from concourse.bass_utils import run_bass_kernel_spmd

    if "nc" not in _cached:
        _cached["nc"] = _build_bass()
    nc = _cached["nc"]

    in_maps = _pack_inputs(x, diag, u)
    res = run_bass_kernel_spmd(nc, in_maps, core_ids=list(range(NCORES)))
    last_results = res

    x2 = np.asarray(x, dtype=np.float32).reshape(B, N)
    d2 = np.asarray(diag, dtype=np.float32).reshape(1, N)
    y = d2 * x2
    inv = 1.0 / (USCALE * USCALE)
    for c in range(NCORES):
        c23 = np.asarray(res.results[c]["corr23"]).astype(np.float32)
        c01 = np.asarray(res.results[c]["corr01"]).astype(np.float32)
        y[:, c * M : (c + 1) * M] += (c23 + c01) * inv
    return y.reshape(B, N, 1).astype(np.float32)



# revision 27
# speedup vs baseline: 1.0638x; 1.0638x over previous
"""HODLR matvec kernel for 8 TRN2 NeuronCores (Bass/Tile).

Sharding: node axis split into 8 contiguous slices of 32768 nodes.
Per core:
  projection  t[l,r,b] = sum_c u[l,c,r] * x[b,c]   (per block, all 8 levels)
              in two passes: level-pairs (0,1) then (2,3), so the
              cross-core collective overlaps the second pass
  tree        combine L7-block partials up to coarser blocks
  AllGather   exchange levels 0-2 partials across cores; the masked
              sibling combine runs on gpsimd once the CC lands
  expansion   corr[b,n] = sum_{l,r} u[l,n,r] * t_sib[l,r,b], split into
              two independent passes: levels 4-7 (local statf only, can
              never wait on the collective) and levels 0-3 (runs after
              the collective, paced by the later ut01 stream); each
              writes its own fp8 output and the host sums them.

Scheduling notes (learned from perfetto traces):
  - All big input streams share the SYNC HWDGE queue in priority order
    x -> u(ph0) -> u(ph1) -> ut23 -> ut01, so the DMA pipe never idles.
  - Tile round-robins every HWDGE DMA over 8 global completion-lane
    semaphores; a late-completing DMA blocks the DMA 8 positions later.
    The collective recv DMAs are therefore emitted BETWEEN the ut23 and
    ut01 streams (they complete right when statf01 is needed), and the
    corr writes go out via gpsimd SWDGE lanes.
  - Engine queues are strict FIFO: everything that waits on the
    collective (recv combine, statf01) lives on gpsimd, which carries
    no other work; nc.any is never used (the scheduler may park ops on
    gpsimd ahead of the CC trigger, delaying it by tens of us).
u/x are fed as fp8e4m3 (u scaled by USCALE; host divides the returned
corrections by USCALE^2). Projection matmuls are fp8 (FWL); expansion
matmuls are fp8 DoubleRow contracting two levels (K=256) per
instruction. Host computes diag*x in fp32 and adds the corrections.
"""

import os
import sys

sys.path.insert(0, "/opt/trn_rl_repo")

import numpy as np
import ml_dtypes

BF16 = ml_dtypes.bfloat16
FP8 = ml_dtypes.float8_e4m3

B = 64
N = 262144
NCORES = 8
M = N // NCORES          # 32768 nodes per core
R = 64
DEPTH = 8
CH = M // 128            # 256 chunks of 128 nodes
NB7 = M // 1024          # 32 L7 blocks (1024 nodes each)
USCALE = 64.0            # u is fed as u*USCALE in fp8 (e4m3 max finite 240)

_cached = {}


def _build_bass():
    import concourse.bacc as bacc
    import concourse.tile as tile
    import concourse.mybir as mybir
    from contextlib import ExitStack

    BF = mybir.dt.bfloat16
    F8 = mybir.dt.float8e4
    F32 = mybir.dt.float32
    ADD = mybir.AluOpType.add
    MULT = mybir.AluOpType.mult

    nc = bacc.Bacc(
        "TRN2",
        target_bir_lowering=False,
        debug=False,
        enable_asserts=False,
        num_devices=NCORES,
    )

    xt_d = nc.dram_tensor("xt", [128, CH, B], F8, kind="ExternalInput").ap()
    # u packed phase-major: [:, ph, k, :] holds level-pairs (2ph, 2ph+1)
    u_d = nc.dram_tensor("u", [128, 2, CH, 256], F8, kind="ExternalInput").ap()
    ut_d = nc.dram_tensor("ut", [4, 128, M], F8, kind="ExternalInput").ap()
    mA_d = nc.dram_tensor("maskA", [128, 8, B], BF, kind="ExternalInput").ap()
    mB_d = nc.dram_tensor("maskB", [64, 8, B], BF, kind="ExternalInput").ap()
    corr23_d = nc.dram_tensor("corr23", [B, M], F8, kind="ExternalOutput").ap()
    corr01_d = nc.dram_tensor("corr01", [B, M], F8, kind="ExternalOutput").ap()

    with tile.TileContext(nc) as tc, ExitStack() as ctx:
        const = ctx.enter_context(tc.tile_pool(name="const", bufs=1))
        upool = ctx.enter_context(tc.tile_pool(name="upool", bufs=4))
        pp = ctx.enter_context(tc.tile_pool(name="pp", bufs=3, space="PSUM"))
        treep = ctx.enter_context(tc.tile_pool(name="treep", bufs=1))
        statp = ctx.enter_context(tc.tile_pool(name="statp", bufs=1))
        ut23p = ctx.enter_context(tc.tile_pool(name="ut23p", bufs=4))
        ut01p = ctx.enter_context(tc.tile_pool(name="ut01p", bufs=8))
        ep23 = ctx.enter_context(tc.tile_pool(name="ep23", bufs=3, space="PSUM"))
        ep01 = ctx.enter_context(tc.tile_pool(name="ep01", bufs=2, space="PSUM"))
        y23p = ctx.enter_context(tc.tile_pool(name="y23p", bufs=3))
        y01p = ctx.enter_context(tc.tile_pool(name="y01p", bufs=2))
        dram = ctx.enter_context(tc.tile_pool(name="dram", bufs=1, space="DRAM"))

        # All big input streams go on the SYNC queue in priority order:
        # xt -> ph0 u -> ph1 u -> ut.  A single queue spreads across all
        # 16 SDMA engines (full BW) and guarantees the stream order, so
        # the expansion's ut stream starts the moment u finishes instead
        # of after the collective (the baseline's ~45us DMA dead zone).
        xt = const.tile([128, CH, B], F8, tag="xt")
        for xq in range(4):
            nc.sync.dma_start(
                xt[:, 64 * xq : 64 * (xq + 1), :],
                xt_d[:, 64 * xq : 64 * (xq + 1), :],
            )
        mA = const.tile([128, 8, B], BF, tag="mA")
        nc.scalar.dma_start(mA[:], mA_d[:])
        mB = const.tile([64, 8, B], BF, tag="mB")
        nc.scalar.dma_start(mB[:], mB_d[:])
        b_in = dram.tile([192, B], BF, tag="b_in")
        b_out = dram.tile([8, 192, B], BF, tag="b_out", addr_space="Shared")

        # ---------------- projection + per-pair trees ----------------
        # G[(q, sz)][j]: [128, 64] bf16; rows 0:64 -> level 2q, rows
        # 64:128 -> level 2q+1 of the j-th block of `sz` nodes (t^T
        # layout [(l,r), b], scaled by USCALE).
        need_top = {3: 2048, 2: 8192, 1: 32768, 0: 32768}
        G = {}

        # Round-robin PSUM drains / casts across Vector, Scalar(ACT) and
        # scheduler-assigned to keep any single engine off the critical
        # path (the baseline had Vector at ~50% busy).
        _cc = [0]

        def tcopy(i, out, in_, engines="vs"):
            # strictly explicit engines: the `any` scheduler can park ops
            # on gpsimd AHEAD of the collective trigger (strict FIFO),
            # delaying the whole collective by tens of us.
            e = engines[_cc[0] % len(engines)]
            _cc[0] += 1
            if e == "v":
                nc.vector.tensor_copy(out, in_)
            else:
                nc.scalar.copy(out, in_)

        _ca = [0]

        def tadd(i, out, a, b):
            nc.vector.tensor_tensor(out, a, b, op=ADD)

        for ph, qs in ((0, (0, 1)), (1, (2, 3))):
            psums = {}  # (q, j) -> psum tile kept for direct tree combine
            for j in range(NB7):
                if j % 4 == 0:
                    u_t2 = upool.tile(
                        [128, 32, 256], F8, tag="u_in", name=f"u_t{ph}_{j}"
                    )
                    if ph == 0 and j == 0:
                        # split the first tile so MM #1 only waits on a
                        # quarter-tile DMA (earlier PE start)
                        for sq in range(4):
                            nc.sync.dma_start(
                                u_t2[:, 8 * sq : 8 * sq + 8, :],
                                u_d[:, ph, 8 * sq : 8 * sq + 8, :],
                            )
                    else:
                        nc.sync.dma_start(
                            u_t2[:], u_d[:, ph, 8 * j : 8 * j + 32, :]
                        )
                u_t = u_t2[:, 8 * (j % 4) : 8 * (j % 4) + 8, :]
                engset = "vs"
                for qi, q in enumerate(qs):
                    ps = pp.tile([128, B], F32, tag="proj", name=f"ps{q}_{j}")
                    for ki in range(8):
                        k = 8 * j + ki
                        nc.tensor.matmul(
                            ps[:],
                            u_t[:, ki, 128 * qi : 128 * qi + 128],
                            xt[:, k, :],
                            start=(ki == 0),
                            stop=(ki == 7),
                        )
                    if q == 3:
                        # pair 3 needs the 1024-blocks themselves
                        g = treep.tile([128, B], BF, tag=f"G3_1024_{j}")
                        tcopy(j, g[:], ps[:], engset)
                        G.setdefault((3, 1024), []).append(g)
                    else:
                        if j % 2 == 0:
                            # drain even block to SBUF (an op may read at
                            # most one PSUM input)
                            tmp = treep.tile(
                                [128, B], BF, tag=f"T{q}_{j // 2}",
                                name=f"T{q}_{j // 2}",
                            )
                            tcopy(j // 2 + q, tmp[:], ps[:], engset)
                            psums[(q, j)] = tmp
                        else:
                            g2 = treep.tile(
                                [128, B], BF, tag=f"G{q}_2048_{j // 2}"
                            )
                            tadd(
                                j // 2,
                                g2[:],
                                psums.pop((q, j - 1))[:],
                                ps[:],
                            )
                            G.setdefault((q, 2048), []).append(g2)
            # tree up
            for q in qs:
                if q == 3:
                    lst = G[(3, 1024)]
                    nxt = []
                    for j in range(len(lst) // 2):
                        g2 = treep.tile([128, B], BF, tag=f"G3_2048_{j}")
                        tadd(j, g2[:], lst[2 * j][:], lst[2 * j + 1][:])
                        nxt.append(g2)
                    G[(3, 2048)] = nxt
                sz = 2048
                while sz < need_top[q]:
                    lst = G[(q, sz)]
                    nxt = []
                    for j in range(len(lst) // 2):
                        g2 = treep.tile([128, B], BF, tag=f"G{q}_{2 * sz}_{j}")
                        tadd(j + q, g2[:], lst[2 * j][:], lst[2 * j + 1][:])
                        nxt.append(g2)
                    G[(q, 2 * sz)] = nxt
                    sz *= 2

            if ph == 0:
                # ------------- collective (overlaps phase 1) -------------
                # AllGather the raw level-0..2 partials; the receive-side
                # masked combine happens later (on gpsimd), once the CC
                # lands.  The b_in copies are plain contiguous SBUF->DRAM
                # writes (fast); the trigger sits on the gpsimd queue.
                A = G[(0, 32768)][0]
                Bt = G[(1, 32768)][0]
                nc.scalar.dma_start(b_in[0:128, :], A[:])
                nc.scalar.dma_start(b_in[128:192, :], Bt[0:64, :])
                nc.gpsimd.collective_compute(
                    "AllGather",
                    mybir.AluOpType.bypass,
                    replica_groups=[list(range(NCORES))],
                    ins=[b_in.opt()],
                    outs=[b_out.opt()],
                )

        # ---- ut streams on the sync queue: ut23 (levels 4-7) first,
        # then the collective recv DMAs (their HWDGE completion-lane
        # slots land between the two streams, completing right when the
        # statf01 pass needs them), then ut01 (levels 0-3).
        ut23_tiles = []
        for gb in range(8):
            t_ = ut23p.tile([128, 2, 4096], F8, tag="ut23", name=f"ut23_{gb}")
            for ff in range(2):
                nc.sync.dma_start(
                    t_[:, ff, :],
                    ut_d[2 + ff, :, 4096 * gb : 4096 * (gb + 1)],
                )
            ut23_tiles.append(t_)

        # collective receive (16 small fast DMAs; they wait on the CC)
        recvA = statp.tile([128, 8, B], BF, tag="recvA")
        recvB = statp.tile([64, 8, B], BF, tag="recvB")
        for k in range(8):
            nc.scalar.dma_start(recvA[:, k, :], b_out[k, 0:128, :])
            nc.scalar.dma_start(recvB[:, k, :], b_out[k, 128:192, :])

        ut01_tiles = []
        for gb in range(8):
            t_ = ut01p.tile([128, 2, 4096], F8, tag="ut01", name=f"ut01_{gb}")
            for ff in range(2):
                nc.sync.dma_start(
                    t_[:, ff, :], ut_d[ff, :, 4096 * gb : 4096 * (gb + 1)]
                )
            ut01_tiles.append(t_)

        # ---------------- expansion stationaries (fp8, DoubleRow) --------
        # statf[q-pair-fuse] tiles [128, 2, B]: [:, j, :] holds the
        # stationary of level-pair (2*fuse + j); rows 0:64 = t_sib at the
        # even level of that pair, rows 64:128 = at the odd level.
        # statf23 first: it has only local dependencies, so the
        # 23-pass matmuls can fire as soon as ut23 arrives.
        statf23 = []
        for m7 in range(NB7):
            s = statp.tile([128, 2, B], F8, tag=f"sf23_{m7}", name=f"sf23_{m7}")
            m5 = m7 // 4
            tcopy(0, s[0:64, 0, :], G[(2, 8192)][(m5 // 2) ^ 1][0:64, :], "vs")
            tcopy(0, s[64:128, 0, :], G[(2, 4096)][m5 ^ 1][64:128, :], "vs")
            tcopy(0, s[0:64, 1, :], G[(3, 2048)][(m7 // 2) ^ 1][0:64, :], "vs")
            tcopy(0, s[64:128, 1, :], G[(3, 1024)][m7 ^ 1][64:128, :], "vs")
            statf23.append(s)

        # ---- masked combine + statf01 (all on gpsimd: it is idle and
        # everything here waits on the collective anyway) ----
        mskA = statp.tile([128, 8, B], BF, tag="mskA")
        mskB = statp.tile([64, 8, B], BF, tag="mskB")
        nc.gpsimd.tensor_tensor(mskA[:], recvA[:], mA[:], op=MULT)
        nc.gpsimd.tensor_tensor(mskB[:], recvB[:], mB[:], op=MULT)
        nc.gpsimd.tensor_tensor(
            mskA[:, 0:4, :], mskA[:, 0:4, :], mskA[:, 4:8, :], op=ADD
        )
        nc.gpsimd.tensor_tensor(
            mskB[:, 0:4, :], mskB[:, 0:4, :], mskB[:, 4:8, :], op=ADD
        )
        nc.gpsimd.tensor_tensor(
            mskA[:, 0:2, :], mskA[:, 0:2, :], mskA[:, 2:4, :], op=ADD
        )
        nc.gpsimd.tensor_tensor(
            mskB[:, 0:2, :], mskB[:, 0:2, :], mskB[:, 2:4, :], op=ADD
        )
        tallA = statp.tile([128, B], BF, tag="tallA")
        tallB = statp.tile([64, B], BF, tag="tallB")
        nc.gpsimd.tensor_tensor(tallA[:], mskA[:, 0, :], mskA[:, 1, :], op=ADD)
        nc.gpsimd.tensor_tensor(tallB[:], mskB[:, 0, :], mskB[:, 1, :], op=ADD)
        statf01 = []
        for m3 in range(2):
            s = statp.tile([128, 2, B], F8, tag=f"sf01_{m3}", name=f"sf01_{m3}")
            nc.gpsimd.tensor_copy(s[:, 0, :], tallA[:])
            nc.gpsimd.tensor_copy(s[0:64, 1, :], tallB[:])
            nc.gpsimd.tensor_copy(
                s[64:128, 1, :], G[(1, 16384)][m3 ^ 1][64:128, :]
            )
            statf01.append(s)

        # ---------------- expansion pass 1: levels 4-7 ----------------
        # Self-contained accumulation groups (start+stop in one MM), so
        # nothing here ever waits on the collective.
        DR = mybir.MatmulPerfMode.DoubleRow
        for gb in range(8):
            ut_t = ut23_tiles[gb]
            y_sb = y23p.tile([B, 4096], F8, tag="y23")
            for gg in range(8):
                g = 8 * gb + gg
                eps = ep23.tile([B, 512], F32, tag="e23", name=f"e23_{g}")
                sl = slice(512 * gg, 512 * (gg + 1))
                nc.tensor.matmul(
                    eps[:], statf23[g // 2][:], ut_t[:, :, sl],
                    start=True, stop=True, perf_mode=DR,
                )
                tcopy(g, y_sb[:, sl], eps[:])
            nc.gpsimd.dma_start(
                corr23_d[:, 4096 * gb : 4096 * (gb + 1)], y_sb[:]
            )

        # ---------------- expansion pass 2: levels 0-3 ----------------
        for gb in range(8):
            ut_t = ut01_tiles[gb]
            y_sb = y01p.tile([B, 4096], F8, tag="y01")
            for gg in range(8):
                g = 8 * gb + gg
                eps = ep01.tile([B, 512], F32, tag="e01", name=f"e01_{g}")
                sl = slice(512 * gg, 512 * (gg + 1))
                nc.tensor.matmul(
                    eps[:], statf01[g // 32][:], ut_t[:, :, sl],
                    start=True, stop=True, perf_mode=DR,
                )
                tcopy(g, y_sb[:, sl], eps[:])
            nc.gpsimd.dma_start(
                corr01_d[:, 4096 * gb : 4096 * (gb + 1)], y_sb[:]
            )

    nc.compile()
    return nc


def _pack_inputs(x, diag, u):
    """Build per-core input maps. x (B,N,1) f32, u (DEPTH,N,R) f32."""
    in_maps = []
    x2 = np.asarray(x).reshape(B, N)
    u3 = np.asarray(u)
    for c in range(NCORES):
        base = c * M
        xsl = x2[:, base : base + M]                      # (B, M)
        usl = u3[:, base : base + M, :] * USCALE          # (8, M, 64)
        xt = np.ascontiguousarray(
            xsl.T.reshape(CH, 128, B).transpose(1, 0, 2)
        ).astype(FP8)                                     # [128, CH, B]
        up = np.ascontiguousarray(
            usl.transpose(1, 0, 2).reshape(M, 512)        # [n, l*64+r]
            .reshape(CH, 128, 2, 256)
            .transpose(1, 2, 0, 3)
        ).astype(FP8)                                     # [128, 2, CH, 256]
        utp = np.ascontiguousarray(
            usl.transpose(0, 2, 1).reshape(4, 128, M)
        ).astype(FP8)                                     # [4, 128, M]
        # masks: mask[d, l] = 1 iff this core c is in the level-l sibling
        # block of destination core d.
        mA = np.zeros((128, 8, B), dtype=BF16)
        mB = np.zeros((64, 8, B), dtype=BF16)
        for d in range(8):
            if (c // 4) == ((d // 4) ^ 1):
                mA[0:64, d, :] = 1.0   # level 0
            if (c // 2) == ((d // 2) ^ 1):
                mA[64:128, d, :] = 1.0  # level 1
            if c == d ^ 1:
                mB[:, d, :] = 1.0       # level 2
        in_maps.append({"xt": xt, "u": up, "ut": utp, "maskA": mA, "maskB": mB})
    return in_maps


last_results = None


def kernel(x, diag, u):
    global last_results
    from concourse.bass_utils import run_bass_kernel_spmd

    if "nc" not in _cached:
        _cached["nc"] = _build_bass()
    nc = _cached["nc"]

    in_maps = _pack_inputs(x, diag, u)
    res = run_bass_kernel_spmd(nc, in_maps, core_ids=list(range(NCORES)))
    last_results = res

    x2 = np.asarray(x, dtype=np.float32).reshape(B, N)
    d2 = np.asarray(diag, dtype=np.float32).reshape(1, N)
    y = d2 * x2
    inv = 1.0 / (USCALE * USCALE)
    for c in range(NCORES):
        c23 = np.asarray(res.results[c]["corr23"]).astype(np.float32)
        c01 = np.asarray(res.results[c]["corr01"]).astype(np.float32)
        y[:, c * M : (c + 1) * M] += (c23 + c01) * inv
    return y.reshape(B, N, 1).astype(np.float32)



# revision 31
# speedup vs baseline: 1.1718x; 1.1015x over previous
"""HODLR matvec kernel for 8 TRN2 NeuronCores (Bass/Tile).

Sharding: node axis split into 8 contiguous slices of 32768 nodes.
Per core:
  projection  t[l,r,b] = sum_c u[l,c,r] * x[b,c]   (per block, all 8 levels)
              in two passes: level-pairs (0,1) then (2,3), so the
              cross-core collective overlaps the second pass
  tree        combine L7-block partials up to coarser blocks
  AllGather   exchange levels 0-2 partials across cores; the masked
              sibling combine runs on gpsimd once the CC lands
  expansion   corr[b,n] = sum_{l,r} u[l,n,r] * t_sib[l,r,b] via fp8
              DoubleRow matmuls, two per 512-node group (levels 4-7
              stationary + levels 0-3 stationary into one PSUM group).

Scheduling notes (learned from perfetto traces):
  - All big input streams share the SYNC HWDGE queue in priority order
    x -> u(ph0) -> u(ph1) -> ut, so the DMA pipe never idles (the
    original kernel started the 16.7MB ut stream only after the
    collective, leaving a ~45us DMA dead zone).
  - The masked sibling combine runs on gpsimd right after the CC:
    engine queues are strict FIFO, so any collective-dependent op on
    vector would block the ph1 drains (and thus the projection).
u/x are fed as fp8e4m3 (u scaled by USCALE; host divides the returned
corrections by USCALE^2). Projection matmuls are fp8 (FWL); expansion
matmuls are fp8 DoubleRow contracting two levels (K=256) per
instruction. Host computes diag*x in fp32 and adds the corrections.
"""

import os
import sys

sys.path.insert(0, "/opt/trn_rl_repo")

import numpy as np
import ml_dtypes

BF16 = ml_dtypes.bfloat16
FP8 = ml_dtypes.float8_e4m3

B = 64
N = 262144
NCORES = 8
M = N // NCORES          # 32768 nodes per core
R = 64
DEPTH = 8
CH = M // 128            # 256 chunks of 128 nodes
NB7 = M // 1024          # 32 L7 blocks (1024 nodes each)
USCALE = 64.0            # u is fed as u*USCALE in fp8 (e4m3 max finite 240)

_cached = {}


def _build_bass():
    import concourse.bacc as bacc
    import concourse.tile as tile
    import concourse.mybir as mybir
    from contextlib import ExitStack

    BF = mybir.dt.bfloat16
    F8 = mybir.dt.float8e4
    F32 = mybir.dt.float32
    ADD = mybir.AluOpType.add
    MULT = mybir.AluOpType.mult

    nc = bacc.Bacc(
        "TRN2",
        target_bir_lowering=False,
        debug=False,
        enable_asserts=False,
        num_devices=NCORES,
    )

    xt_d = nc.dram_tensor("xt", [128, CH, B], F8, kind="ExternalInput").ap()
    # u packed phase-major: [:, ph, k, :] holds level-pairs (2ph, 2ph+1)
    u_d = nc.dram_tensor("u", [128, 2, CH, 256], F8, kind="ExternalInput").ap()
    ut_d = nc.dram_tensor("ut", [4, 128, M], F8, kind="ExternalInput").ap()
    mA_d = nc.dram_tensor("maskA", [128, 8, B], BF, kind="ExternalInput").ap()
    mB_d = nc.dram_tensor("maskB", [64, 8, B], BF, kind="ExternalInput").ap()
    corr_d = nc.dram_tensor("corr", [B, M], F8, kind="ExternalOutput").ap()

    with tile.TileContext(nc) as tc, ExitStack() as ctx:
        const = ctx.enter_context(tc.tile_pool(name="const", bufs=1))
        upool = ctx.enter_context(tc.tile_pool(name="upool", bufs=5))
        pp = ctx.enter_context(tc.tile_pool(name="pp", bufs=4, space="PSUM"))
        treep = ctx.enter_context(tc.tile_pool(name="treep", bufs=1))
        statp = ctx.enter_context(tc.tile_pool(name="statp", bufs=1))
        utp = ctx.enter_context(tc.tile_pool(name="utp", bufs=5))
        ep = ctx.enter_context(tc.tile_pool(name="ep", bufs=4, space="PSUM"))
        yp = ctx.enter_context(tc.tile_pool(name="yp", bufs=3))
        dram = ctx.enter_context(tc.tile_pool(name="dram", bufs=1, space="DRAM"))

        # All big input streams go on the SYNC queue in priority order:
        # xt -> ph0 u -> ph1 u -> ut.  A single queue spreads across all
        # 16 SDMA engines (full BW) and guarantees the stream order, so
        # the expansion's ut stream starts the moment u finishes instead
        # of after the collective (the baseline's ~45us DMA dead zone).
        xt = const.tile([128, CH, B], F8, tag="xt")
        for xq in range(4):
            nc.sync.dma_start(
                xt[:, 64 * xq : 64 * (xq + 1), :],
                xt_d[:, 64 * xq : 64 * (xq + 1), :],
            )
        mA = const.tile([128, 8, B], BF, tag="mA")
        nc.scalar.dma_start(mA[:], mA_d[:])
        mB = const.tile([64, 8, B], BF, tag="mB")
        nc.scalar.dma_start(mB[:], mB_d[:])
        b_in = dram.tile([192, B], BF, tag="b_in")
        b_out = dram.tile([8, 192, B], BF, tag="b_out", addr_space="Shared")

        # ---------------- projection + per-pair trees ----------------
        # G[(q, sz)][j]: [128, 64] bf16; rows 0:64 -> level 2q, rows
        # 64:128 -> level 2q+1 of the j-th block of `sz` nodes (t^T
        # layout [(l,r), b], scaled by USCALE).
        need_top = {3: 2048, 2: 8192, 1: 32768, 0: 32768}
        G = {}

        # Round-robin PSUM drains / casts across Vector, Scalar(ACT) and
        # scheduler-assigned to keep any single engine off the critical
        # path (the baseline had Vector at ~50% busy).
        _cc = [0]

        def tcopy(i, out, in_, engines="vsa"):
            e = engines[_cc[0] % len(engines)]
            _cc[0] += 1
            if e == "v":
                nc.vector.tensor_copy(out, in_)
            elif e == "s":
                nc.scalar.copy(out, in_)
            else:
                nc.any.tensor_copy(out, in_)

        _ca = [0]

        def tadd(i, out, a, b):
            if _ca[0] % 2 == 0:
                nc.vector.tensor_tensor(out, a, b, op=ADD)
            else:
                nc.any.tensor_add(out, a, b)
            _ca[0] += 1

        for ph, qs in ((0, (0, 1)), (1, (2, 3))):
            psums = {}  # (q, j) -> psum tile kept for direct tree combine
            for j in range(NB7):
                if j % 4 == 0:
                    u_t2 = upool.tile(
                        [128, 32, 256], F8, tag="u_in", name=f"u_t{ph}_{j}"
                    )
                    if ph == 0 and j == 0:
                        # split the first tile so MM #1 only waits on a
                        # quarter-tile DMA (earlier PE start)
                        for sq in range(4):
                            nc.sync.dma_start(
                                u_t2[:, 8 * sq : 8 * sq + 8, :],
                                u_d[:, ph, 8 * sq : 8 * sq + 8, :],
                            )
                    else:
                        nc.sync.dma_start(
                            u_t2[:], u_d[:, ph, 8 * j : 8 * j + 32, :]
                        )
                u_t = u_t2[:, 8 * (j % 4) : 8 * (j % 4) + 8, :]
                engset = "vsa" if ph == 0 else "va"
                for qi, q in enumerate(qs):
                    ps = pp.tile([128, B], F32, tag="proj", name=f"ps{q}_{j}")
                    for ki in range(8):
                        k = 8 * j + ki
                        nc.tensor.matmul(
                            ps[:],
                            u_t[:, ki, 128 * qi : 128 * qi + 128],
                            xt[:, k, :],
                            start=(ki == 0),
                            stop=(ki == 7),
                        )
                    if q == 3:
                        # pair 3 needs the 1024-blocks themselves
                        g = treep.tile([128, B], BF, tag=f"G3_1024_{j}")
                        tcopy(j, g[:], ps[:], engset)
                        G.setdefault((3, 1024), []).append(g)
                    else:
                        if j % 2 == 0:
                            # drain even block to SBUF (an op may read at
                            # most one PSUM input)
                            tmp = treep.tile(
                                [128, B], BF, tag=f"T{q}_{j // 2}",
                                name=f"T{q}_{j // 2}",
                            )
                            tcopy(j // 2 + q, tmp[:], ps[:], engset)
                            psums[(q, j)] = tmp
                        else:
                            g2 = treep.tile(
                                [128, B], BF, tag=f"G{q}_2048_{j // 2}"
                            )
                            tadd(
                                j // 2,
                                g2[:],
                                psums.pop((q, j - 1))[:],
                                ps[:],
                            )
                            G.setdefault((q, 2048), []).append(g2)
            # tree up
            for q in qs:
                if q == 3:
                    lst = G[(3, 1024)]
                    nxt = []
                    for j in range(len(lst) // 2):
                        g2 = treep.tile([128, B], BF, tag=f"G3_2048_{j}")
                        tadd(j, g2[:], lst[2 * j][:], lst[2 * j + 1][:])
                        nxt.append(g2)
                    G[(3, 2048)] = nxt
                sz = 2048
                while sz < need_top[q]:
                    lst = G[(q, sz)]
                    nxt = []
                    for j in range(len(lst) // 2):
                        g2 = treep.tile([128, B], BF, tag=f"G{q}_{2 * sz}_{j}")
                        tadd(j + q, g2[:], lst[2 * j][:], lst[2 * j + 1][:])
                        nxt.append(g2)
                    G[(q, 2 * sz)] = nxt
                    sz *= 2

            if ph == 0:
                # ------------- collective (overlaps phase 1) -------------
                # AllGather the raw level-0..2 partials; the receive-side
                # masked combine happens later (on gpsimd), once the CC
                # lands.  The b_in copies are plain contiguous SBUF->DRAM
                # writes (fast); the trigger sits on the gpsimd queue.
                A = G[(0, 32768)][0]
                Bt = G[(1, 32768)][0]
                nc.scalar.dma_start(b_in[0:128, :], A[:])
                nc.scalar.dma_start(b_in[128:192, :], Bt[0:64, :])
                nc.gpsimd.collective_compute(
                    "AllGather",
                    mybir.AluOpType.bypass,
                    replica_groups=[list(range(NCORES))],
                    ins=[b_in.opt()],
                    outs=[b_out.opt()],
                )
                # The entire receive path runs on engines with NO ph1
                # work queued (scalar DMAs + gpsimd ALU): engine queues
                # are strict FIFO, so putting any of this on vector/any
                # would block the ph1 drains (and thus the projection)
                # until the collective lands.
                recvA = statp.tile([128, 8, B], BF, tag="recvA")
                recvB = statp.tile([64, 8, B], BF, tag="recvB")
                for k in range(8):
                    nc.scalar.dma_start(recvA[:, k, :], b_out[k, 0:128, :])
                    nc.scalar.dma_start(recvB[:, k, :], b_out[k, 128:192, :])
                mskA = statp.tile([128, 8, B], BF, tag="mskA")
                mskB = statp.tile([64, 8, B], BF, tag="mskB")
                nc.gpsimd.tensor_tensor(mskA[:], recvA[:], mA[:], op=MULT)
                nc.gpsimd.tensor_tensor(mskB[:], recvB[:], mB[:], op=MULT)
                # fold-halves reduction over the 8 cores (contiguous APs)
                nc.gpsimd.tensor_tensor(
                    mskA[:, 0:4, :], mskA[:, 0:4, :], mskA[:, 4:8, :], op=ADD
                )
                nc.gpsimd.tensor_tensor(
                    mskB[:, 0:4, :], mskB[:, 0:4, :], mskB[:, 4:8, :], op=ADD
                )
                nc.gpsimd.tensor_tensor(
                    mskA[:, 0:2, :], mskA[:, 0:2, :], mskA[:, 2:4, :], op=ADD
                )
                nc.gpsimd.tensor_tensor(
                    mskB[:, 0:2, :], mskB[:, 0:2, :], mskB[:, 2:4, :], op=ADD
                )
                tallA = statp.tile([128, B], BF, tag="tallA")
                tallB = statp.tile([64, B], BF, tag="tallB")
                nc.gpsimd.tensor_tensor(
                    tallA[:], mskA[:, 0, :], mskA[:, 1, :], op=ADD
                )
                nc.gpsimd.tensor_tensor(
                    tallB[:], mskB[:, 0, :], mskB[:, 1, :], op=ADD
                )

        # ---- expansion ut stream: all loads pre-issued on the sync
        # queue (ordered behind the u stream).  bufs=5 keeps ~10MB of
        # lookahead; tiles 5..7 flow-control on the consumer.
        ut_tiles = []
        for gb in range(8):
            t_ = utp.tile([128, 4, 4096], F8, tag="utf", name=f"ut_{gb}")
            for ff in range(4):
                nc.sync.dma_start(
                    t_[:, ff, :], ut_d[ff, :, 4096 * gb : 4096 * (gb + 1)]
                )
            ut_tiles.append(t_)

        # ---------------- expansion stationaries (fp8, DoubleRow) --------
        # statf[q-pair-fuse] tiles [128, 2, B]: [:, j, :] holds the
        # stationary of level-pair (2*fuse + j); rows 0:64 = t_sib at the
        # even level of that pair, rows 64:128 = at the odd level.
        # statf23 first: it has only local dependencies, so the
        # expansion's start-matmuls can fire as soon as ut arrives.
        statf23 = []
        for m7 in range(NB7):
            s = statp.tile([128, 2, B], F8, tag=f"sf23_{m7}", name=f"sf23_{m7}")
            m5 = m7 // 4
            tcopy(0, s[0:64, 0, :], G[(2, 8192)][(m5 // 2) ^ 1][0:64, :], "va")
            tcopy(0, s[64:128, 0, :], G[(2, 4096)][m5 ^ 1][64:128, :], "va")
            tcopy(0, s[0:64, 1, :], G[(3, 2048)][(m7 // 2) ^ 1][0:64, :], "va")
            tcopy(0, s[64:128, 1, :], G[(3, 1024)][m7 ^ 1][64:128, :], "va")
            statf23.append(s)

        # ---- statf01 (depends on collective result; scalar queue) ----
        statf01 = []
        for m3 in range(2):
            s = statp.tile([128, 2, B], F8, tag=f"sf01_{m3}", name=f"sf01_{m3}")
            nc.scalar.copy(s[:, 0, :], tallA[:])
            nc.scalar.copy(s[0:64, 1, :], tallB[:])
            nc.scalar.copy(s[64:128, 1, :], G[(1, 16384)][m3 ^ 1][64:128, :])
            statf01.append(s)

        # ---------------- expansion (DoubleRow fp8) ----------------
        DR = mybir.MatmulPerfMode.DoubleRow
        for gb in range(8):  # 8 blocks of 8 groups x 512 nodes
            ut_t = ut_tiles[gb]
            y_sb = yp.tile([B, 4096], F8, tag="y")
            for gg in range(8):
                g = 8 * gb + gg
                eps = ep.tile([B, 512], F32, tag="exp", name=f"eps{g}")
                sl = slice(512 * gg, 512 * (gg + 1))
                nc.tensor.matmul(
                    eps[:], statf23[g // 2][:], ut_t[:, 2:4, sl],
                    start=True, stop=False, perf_mode=DR,
                )
                nc.tensor.matmul(
                    eps[:], statf01[g // 32][:], ut_t[:, 0:2, sl],
                    start=False, stop=True, perf_mode=DR,
                )
                tcopy(g, y_sb[:, sl], eps[:])
            # corr writes go on the scalar queue so they overlap the
            # still-draining ut stream on the sync queue.
            nc.scalar.dma_start(corr_d[:, 4096 * gb : 4096 * (gb + 1)], y_sb[:])

    nc.compile()
    return nc


def _pack_inputs(x, diag, u):
    """Build per-core input maps. x (B,N,1) f32, u (DEPTH,N,R) f32."""
    in_maps = []
    x2 = np.asarray(x).reshape(B, N)
    u3 = np.asarray(u)
    for c in range(NCORES):
        base = c * M
        xsl = x2[:, base : base + M]                      # (B, M)
        usl = u3[:, base : base + M, :] * USCALE          # (8, M, 64)
        xt = np.ascontiguousarray(
            xsl.T.reshape(CH, 128, B).transpose(1, 0, 2)
        ).astype(FP8)                                     # [128, CH, B]
        up = np.ascontiguousarray(
            usl.transpose(1, 0, 2).reshape(M, 512)        # [n, l*64+r]
            .reshape(CH, 128, 2, 256)
            .transpose(1, 2, 0, 3)
        ).astype(FP8)                                     # [128, 2, CH, 256]
        utp = np.ascontiguousarray(
            usl.transpose(0, 2, 1).reshape(4, 128, M)
        ).astype(FP8)                                     # [4, 128, M]
        # masks: mask[d, l] = 1 iff this core c is in the level-l sibling
        # block of destination core d.
        mA = np.zeros((128, 8, B), dtype=BF16)
        mB = np.zeros((64, 8, B), dtype=BF16)
        for d in range(8):
            if (c // 4) == ((d // 4) ^ 1):
                mA[0:64, d, :] = 1.0   # level 0
            if (c // 2) == ((d // 2) ^ 1):
                mA[64:128, d, :] = 1.0  # level 1
            if c == d ^ 1:
                mB[:, d, :] = 1.0       # level 2
        in_maps.append({"xt": xt, "u": up, "ut": utp, "maskA": mA, "maskB": mB})
    return in_maps


last_results = None


def kernel(x, diag, u):
    global last_results
    from concourse.bass_utils import run_bass_kernel_spmd

    if "nc" not in _cached:
        _cached["nc"] = _build_bass()
    nc = _cached["nc"]

    in_maps = _pack_inputs(x, diag, u)
    res = run_bass_kernel_spmd(nc, in_maps, core_ids=list(range(NCORES)))
    last_results = res

    x2 = np.asarray(x, dtype=np.float32).reshape(B, N)
    d2 = np.asarray(diag, dtype=np.float32).reshape(1, N)
    y = d2 * x2
    inv = 1.0 / (USCALE * USCALE)
    for c in range(NCORES):
        corr = np.asarray(res.results[c]["corr"]).astype(np.float32)
        y[:, c * M : (c + 1) * M] += corr * inv
    return y.reshape(B, N, 1).astype(np.float32)

